# revision 26
# baseline (speedup 1.0000x reference)
"""DotGAT layer (segment-softmax GNN message passing) on 8 Trainium2 cores.

Strategy (graph/data parallel per the sharding hint):
  - Nodes are split into 8 contiguous ranges of 6272 (49 aligned 128-node
    blocks); each core owns the edges whose dst falls in its range.
  - The halo exchange is done as data layout on the host: each core receives
    a feature-major stream z_e[:, j] = z[src_j].T of its edges' source
    features (edges grouped by dst block, padded to 128-edge chunks).  The
    device projects k|v PER EDGE from that stream (z_e chunk is the matmul
    stationary operand, [Wk|Wv] the moving one) — trading cheap PE flops for
    the per-edge DMA-gather descriptors that otherwise dominate.
  - q is projected on device for the core's own 6272 nodes and kept in SBUF.
  - Per 128-edge chunk, host-streamed one-hot matrices M [node,edge] and
    M^T [edge,node] (fp8, exact 0/1) turn the q-expansion and the
    segment-sum into PE matmuls; a fused DVE affine_mul_reduce computes the
    per-edge logits; ACT computes exp (fp16); a broadcast tensor_tensor
    forms ex*v; the aggregation matmul accumulates num|den in PSUM per
    block.  h = num / den (den==0 -> 0).

The program is recompiled per call with all data-dependent sizes baked in as
compile-time constants; per-core variation lives purely in the input data
(SPMD: one instruction stream, 8 cores).
"""

import sys

sys.path.insert(0, "/opt/trn_rl_repo")

import numpy as np
import ml_dtypes

N_NODES = 50000
DIM = 128
N_CORES = 8
BLK = 128
BLOCKS_PER_CORE = 49
NODES_PER_CORE = BLOCKS_PER_CORE * BLK  # 6272
N_PAD = NODES_PER_CORE * N_CORES  # 50176
TAU = 1.0 / np.sqrt(DIM)

F8 = ml_dtypes.float8_e4m3


def _prepare(z, Wq, bq, Wk, bk, Wv, bv, src, dst):
    """Host-side sharding: per-core edge grouping, one-hot metadata and the
    edge-major source-feature stream (pure data movement, no arithmetic)."""
    z = np.asarray(z, np.float32)
    src = np.asarray(src, np.int32)
    dst = np.asarray(dst, np.int32)

    W_all = np.concatenate(
        [np.asarray(Wq, np.float32), np.asarray(Wk, np.float32), np.asarray(Wv, np.float32)],
        axis=1,
    )  # [128, 384]
    b_all = np.concatenate(
        [np.asarray(bq, np.float32), np.asarray(bk, np.float32), np.asarray(bv, np.float32)]
    )  # [384]
    has_bias = bool(np.any(b_all != 0.0))

    # feature-major z (fp16), one extra zero column for edge padding
    zT = np.zeros((DIM, N_PAD + 1), np.float16)
    zT[:, :N_NODES] = z.T.astype(np.float16)

    per_core = []
    for c in range(N_CORES):
        n0 = c * NODES_PER_CORE
        sel = (dst >= n0) & (dst < n0 + NODES_PER_CORE)
        es = src[sel].astype(np.int64)
        ed = (dst[sel] - n0).astype(np.int64)
        blk = ed >> 7
        order = np.lexsort((ed, blk))
        es, ed, blk = es[order], ed[order], blk[order]
        cnt = np.zeros(BLOCKS_PER_CORE, np.int64)
        np.add.at(cnt, blk, 1)
        per_core.append(dict(es=es, ed=ed, cnt=cnt))

    cnts = np.stack([pc["cnt"] for pc in per_core])  # [8, 49]
    C = (-(-cnts // BLK)).max(axis=0)  # [49] per-position chunk counts
    S = int(C.sum())

    in_maps = []
    for c in range(N_CORES):
        pc = per_core[c]
        es, ed, cnt = pc["es"], pc["ed"], pc["cnt"]
        # per-slot source column list, padded with the zero column
        col = np.full(S * BLK, N_PAD, np.int64)
        meta = np.zeros((128, S * 256), F8)  # per slot: M (128) | M^T (128)
        off = 0
        ptr = 0
        for b in range(BLOCKS_PER_CORE):
            Cc = int(C[b])
            if Cc == 0:
                continue
            n = int(cnt[b])
            col[off * BLK : off * BLK + n] = es[ptr : ptr + n]
            drel = ed[ptr : ptr + n] - b * BLK
            ptr += n
            for cc in range(Cc):
                lo = cc * BLK
                m = min(BLK, n - lo)
                if m <= 0:
                    break
                d = drel[lo : lo + m]
                base = (off + cc) * 256
                Mc = np.zeros((BLK, BLK), np.float32)
                Mc[d, np.arange(m)] = 1.0
                meta[:, base : base + 128] = Mc.astype(F8)
                MTc = np.zeros((BLK, BLK), np.float32)
                MTc[np.arange(m), d] = 1.0
                meta[:, base + 128 : base + 256] = MTc.astype(F8)
            off += Cc
        ze = np.ascontiguousarray(zT[:, col])  # [128, S*128] fp16
        zq = np.ascontiguousarray(
            zT[:, c * NODES_PER_CORE : c * NODES_PER_CORE + NODES_PER_CORE]
        )
        in_maps.append(
            dict(
                ze=ze,
                zq=zq,
                Wall=W_all.astype(np.float16),
                bias=b_all.reshape(1, 384).astype(np.float16),
                meta=meta,
            )
        )
    consts = dict(C=C, S=S, has_bias=has_bias)
    return in_maps, consts


def _build(consts):
    import concourse.bacc as bacc
    import concourse.mybir as mybir
    import concourse.tile as tile

    dt = mybir.dt
    Alu = mybir.AluOpType
    Act = mybir.ActivationFunctionType

    C = consts["C"]
    S = consts["S"]
    has_bias = consts["has_bias"]

    nc = bacc.Bacc("TRN2", target_bir_lowering=False, debug=False, num_devices=N_CORES)

    ze = nc.declare_dram_parameter("ze", [128, S * BLK], dt.float16, isOutput=False)
    zq = nc.declare_dram_parameter("zq", [128, NODES_PER_CORE], dt.float16, isOutput=False)
    Wall = nc.declare_dram_parameter("Wall", [128, 384], dt.float16, isOutput=False)
    bias = nc.declare_dram_parameter("bias", [1, 384], dt.float16, isOutput=False)
    meta = nc.declare_dram_parameter("meta", [128, S * 256], dt.float8e4, isOutput=False)
    h = nc.declare_dram_parameter("h", [NODES_PER_CORE, DIM], dt.float32, isOutput=True)

    with tile.TileContext(nc) as tc:
        with (
            tc.tile_pool(name="const", bufs=1) as constp,
            tc.tile_pool(name="qbuf", bufs=1) as qbuf,
        ):
            wall_sb = constp.tile([128, 384], dt.float16)
            nc.sync.dma_start(wall_sb[:], Wall[:])
            if has_bias:
                bias_sb = constp.tile([1, 384], dt.float16)
                ones1 = constp.tile([1, 128], dt.float16)
                nc.sync.dma_start(bias_sb[:], bias[:])
                nc.vector.memset(ones1[:], 1.0)
            q_sb = qbuf.tile([128, BLOCKS_PER_CORE * BLK], dt.float16)

            # ---- PE warm-up: ~9us of dense matmuls so the HAM clock-gate
            # lifts the PE from 1.2 to 2.4 GHz before the main loop ----
            with tc.tile_pool(name="warm", bufs=4, space="PSUM") as wpool:
                for i in range(80):
                    wps = wpool.tile([128, 128], dt.float32, tag="w")
                    nc.tensor.matmul(
                        wps[:], lhsT=wall_sb[:, 0:128], rhs=wall_sb[:, 0:128],
                        start=True, stop=True,
                    )

            # ---- prologue: project q for the core's own blocks ----
            with (
                tc.tile_pool(name="zt", bufs=3) as zpool,
                tc.tile_pool(name="pps", bufs=3, space="PSUM") as ppool,
            ):
                for g in range((BLOCKS_PER_CORE + 3) // 4):  # 4 blocks per DMA
                    lo = g * 4
                    nb = min(4, BLOCKS_PER_CORE - lo)
                    zt = zpool.tile([128, nb * 128], dt.float16, tag="zt")
                    nc.sync.dma_start(
                        zt[:], zq[:, lo * 128 : (lo + nb) * 128]
                    )
                    for i in range(nb):
                        b = lo + i
                        ps = ppool.tile([128, 128], dt.float32, tag="ps")
                        nc.tensor.matmul(
                            ps[:], lhsT=zt[:, i * 128 : (i + 1) * 128],
                            rhs=wall_sb[:, 0:128], start=True, stop=not has_bias,
                        )
                        if has_bias:
                            nc.tensor.matmul(
                                ps[:], lhsT=ones1[:], rhs=bias_sb[:, 0:128],
                                start=False, stop=True,
                            )
                        nc.scalar.copy(q_sb[:, b * 128 : (b + 1) * 128], ps[:])

            # ---- edge phase ----
            with (
                tc.tile_pool(name="zep", bufs=3) as zep,
                tc.tile_pool(name="mp", bufs=3) as mp,
                tc.tile_pool(name="kvp", bufs=10) as kvp,
                tc.tile_pool(name="xp", bufs=8) as xp,
                tc.tile_pool(name="ep", bufs=8) as ep,
                tc.tile_pool(name="scr", bufs=4) as scr,
                tc.tile_pool(name="hp", bufs=2) as hp,
                tc.tile_pool(name="sm", bufs=4) as sm,
                tc.tile_pool(name="pskv", bufs=3, space="PSUM") as pskv,
                tc.tile_pool(name="psq", bufs=2, space="PSUM") as psq,
                tc.tile_pool(name="psnd", bufs=3, space="PSUM") as psnd,
            ):
                offs = np.concatenate([[0], np.cumsum(C)]).astype(int)
                for b in range(BLOCKS_PER_CORE):
                    Cc = int(C[b])
                    off = int(offs[b])
                    ndA = psnd.tile([128, 132], dt.float32, tag="nd")
                    ndB = (
                        psnd.tile([128, 132], dt.float32, tag="nd", name="ndB")
                        if Cc > 1 else None
                    )
                    ht = hp.tile([128, 128], dt.float32, tag="h")
                    if Cc == 0:
                        nc.vector.memset(ht[:], 0.0)
                        nc.sync.dma_start(h[b * 128 : (b + 1) * 128, :], ht[:])
                        continue
                    zs = zep.tile([128, Cc * 128], dt.float16, tag="ze")
                    nc.sync.dma_start(zs[:], ze[:, off * 128 : (off + Cc) * 128])
                    mt_ = mp.tile([128, Cc * 256], dt.float8e4, tag="m")
                    nc.sync.dma_start(mt_[:], meta[:, off * 256 : (off + Cc) * 256])
                    for c0 in range(0, Cc, 2):
                        npair = min(2, Cc - c0)
                        xt2 = xp.tile([128, 2, 132], dt.float16, tag="x")
                        e2 = ep.tile([128, 2], dt.float32, tag="e")
                        kvts = []
                        for j in range(npair):
                            cc = c0 + j
                            # per-edge k|v projection
                            kvps = pskv.tile([128, 256], dt.float32, tag="kvps")
                            nc.tensor.matmul(
                                kvps[:], lhsT=zs[:, cc * 128 : (cc + 1) * 128],
                                rhs=wall_sb[:, 128:384], start=True, stop=not has_bias,
                            )
                            if has_bias:
                                nc.tensor.matmul(
                                    kvps[:], lhsT=ones1[:], rhs=bias_sb[:, 128:384],
                                    start=False, stop=True,
                                )
                            kvt = kvp.tile([128, 256], dt.float16, tag="kvt")
                            if cc % 3 == 0:
                                nc.vector.tensor_copy(kvt[:], kvps[:])
                            else:
                                nc.scalar.copy(kvt[:], kvps[:])
                            kvts.append(kvt)
                            # q expansion to edges
                            qd = psq.tile([128, 128], dt.float32, tag="qd")
                            nc.tensor.matmul(
                                qd[:],
                                lhsT=mt_[:, cc * 256 : cc * 256 + 128],
                                rhs=q_sb[:, b * 128 : (b + 1) * 128],
                                start=True, stop=True,
                            )
                            # e = tau * sum_f k*q_dst  (fused mul+reduce)
                            sct = scr.tile([128, 128], dt.float16, tag="sc")
                            nc.vector.affine_mul_reduce(
                                out=sct[:], accum_out=e2[:, j : j + 1],
                                in0=kvt[:, 0:128], in1=qd[:],
                                scale=float(TAU), bias=0.0,
                            )
                        # one exp per chunk pair, strided into the den column
                        nc.scalar.activation(
                            xt2[:, 0:npair, 128:129],
                            e2[:, 0:npair].rearrange("p (a b) -> p a b", b=1),
                            Act.Exp,
                        )
                        for j in range(npair):
                            cc = c0 + j
                            # x = v * exp(e) on the otherwise-idle GPSIMD engine
                            nc.gpsimd.tensor_tensor(
                                out=xt2[:, j, 0:128],
                                in0=kvts[j][:, 128:256],
                                in1=xt2[:, j, 128:129].broadcast_to((128, 128)),
                                op=Alu.mult,
                            )
                            # segment-sum into num|den; alternate banks so
                            # adjacent accumulates don't serialize on the PE
                            ndx = ndA if j == 0 else ndB
                            nc.tensor.matmul(
                                ndx[:, 0:129],
                                lhsT=mt_[:, cc * 256 + 128 : cc * 256 + 256],
                                rhs=xt2[:, j, 0:129],
                                start=(cc <= 1), stop=(cc >= Cc - 2),
                            )
                    # ---- drain block b: h = num / max(den, den==0) ----
                    nds = sm.tile([128, 132], dt.float32, tag="nds")
                    nc.vector.tensor_copy(nds[:, 0:129], ndA[:, 0:129])
                    if ndB is not None:
                        nc.vector.tensor_tensor(
                            out=nds[:, 0:129], in0=nds[:, 0:129], in1=ndB[:, 0:129],
                            op=Alu.add,
                        )
                    z01 = sm.tile([128, 1], dt.float32, tag="z01")
                    nc.vector.tensor_scalar(
                        out=z01[:], in0=nds[:, 128:129],
                        scalar1=0.0, scalar2=None, op0=Alu.is_equal,
                    )
                    dsafe = sm.tile([128, 1], dt.float32, tag="ds")
                    nc.vector.tensor_tensor(
                        out=dsafe[:], in0=nds[:, 128:129], in1=z01[:], op=Alu.add
                    )
                    rec = sm.tile([128, 1], dt.float32, tag="rec")
                    nc.vector.reciprocal(rec[:], dsafe[:])
                    nc.vector.tensor_scalar(
                        out=ht[:], in0=nds[:, 0:128],
                        scalar1=rec[:], scalar2=None, op0=Alu.mult,
                    )
                    nc.sync.dma_start(h[b * 128 : (b + 1) * 128, :], ht[:])

    nc.compile()
    return nc


def _install_ntff_hook():
    """The agent image's antenv lacks axon_hooks; recreate it and register
    the ctypes NTFF profile hook the boot would have installed."""
    import types

    if "antenv.axon_hooks" not in sys.modules:
        import antenv

        m = types.ModuleType("antenv.axon_hooks")
        m._hook = None
        m.set_axon_ntff_profile_hook = lambda h, _m=m: setattr(_m, "_hook", h)
        m.get_axon_ntff_profile_hook = lambda _m=m: _m._hook
        sys.modules["antenv.axon_hooks"] = m
        antenv.axon_hooks = m
    from antenv import axon_hooks

    if axon_hooks.get_axon_ntff_profile_hook() is None:
        from trn_agent_boot.trn_boot import _ntff_profile_via_ctypes

        hook = _ntff_profile_via_ctypes("/opt/axon/libaxon_pjrt.so")
        if hook is not None:
            axon_hooks.set_axon_ntff_profile_hook(hook)


def run(inputs, trace=False):
    """Returns (h [50000,128] float32, exec_time_ns or None)."""
    from concourse.bass_utils import run_bass_kernel_spmd

    if trace:
        try:
            _install_ntff_hook()
        except Exception as e:  # profiling is best-effort
            print(f"ntff hook install failed: {e}", file=sys.stderr)

    in_maps, consts = _prepare(**inputs)
    nc = _build(consts)
    res = run_bass_kernel_spmd(
        nc,
        [dict(m) for m in in_maps],
        list(range(N_CORES)),
        trace=trace,
    )
    h = np.concatenate([r["h"] for r in res.results], axis=0)[:N_NODES]
    return np.ascontiguousarray(h.astype(np.float32)), res.exec_time_ns


def kernel(**inputs) -> np.ndarray:
    h, _ = run(inputs, trace=False)
    return h


# revision 28
# speedup vs baseline: 1.0177x; 1.0177x over previous
"""DotGAT layer (segment-softmax GNN message passing) on 8 Trainium2 cores.

Strategy (graph/data parallel per the sharding hint):
  - Nodes are split into 8 contiguous ranges of 6272 (49 aligned 128-node
    blocks); each core owns the edges whose dst falls in its range.
  - The halo exchange is done as data layout on the host: each core receives
    a feature-major stream z_e[:, j] = z[src_j].T of its edges' source
    features (edges grouped by dst block, padded to 128-edge chunks).  The
    device projects k|v PER EDGE from that stream (z_e chunk is the matmul
    stationary operand, [Wk|Wv] the moving one) — trading cheap PE flops for
    the per-edge DMA-gather descriptors that otherwise dominate.
  - q is projected on device for the core's own 6272 nodes and kept in SBUF.
  - Per 128-edge chunk, host-streamed one-hot matrices M [node,edge] and
    M^T [edge,node] (fp8, exact 0/1) turn the q-expansion and the
    segment-sum into PE matmuls; a fused DVE affine_mul_reduce computes the
    per-edge logits; ACT computes exp (fp16); a broadcast tensor_tensor
    forms ex*v; the aggregation matmul accumulates num|den in PSUM per
    block.  h = num / den (den==0 -> 0).

The program is recompiled per call with all data-dependent sizes baked in as
compile-time constants; per-core variation lives purely in the input data
(SPMD: one instruction stream, 8 cores).
"""

import sys

sys.path.insert(0, "/opt/trn_rl_repo")

import numpy as np
import ml_dtypes

N_NODES = 50000
DIM = 128
N_CORES = 8
BLK = 128
BLOCKS_PER_CORE = 49
NODES_PER_CORE = BLOCKS_PER_CORE * BLK  # 6272
N_PAD = NODES_PER_CORE * N_CORES  # 50176
TAU = 1.0 / np.sqrt(DIM)

F8 = ml_dtypes.float8_e4m3


def _prepare(z, Wq, bq, Wk, bk, Wv, bv, src, dst):
    """Host-side sharding: per-core edge grouping, one-hot metadata and the
    edge-major source-feature stream (pure data movement, no arithmetic)."""
    z = np.asarray(z, np.float32)
    src = np.asarray(src, np.int32)
    dst = np.asarray(dst, np.int32)

    W_all = np.concatenate(
        [np.asarray(Wq, np.float32), np.asarray(Wk, np.float32), np.asarray(Wv, np.float32)],
        axis=1,
    )  # [128, 384]
    b_all = np.concatenate(
        [np.asarray(bq, np.float32), np.asarray(bk, np.float32), np.asarray(bv, np.float32)]
    )  # [384]
    has_bias = bool(np.any(b_all != 0.0))

    # feature-major z (fp16), one extra zero column for edge padding
    zT = np.zeros((DIM, N_PAD + 1), np.float16)
    zT[:, :N_NODES] = z.T.astype(np.float16)

    per_core = []
    for c in range(N_CORES):
        n0 = c * NODES_PER_CORE
        sel = (dst >= n0) & (dst < n0 + NODES_PER_CORE)
        es = src[sel].astype(np.int64)
        ed = (dst[sel] - n0).astype(np.int64)
        blk = ed >> 7
        order = np.lexsort((ed, blk))
        es, ed, blk = es[order], ed[order], blk[order]
        cnt = np.zeros(BLOCKS_PER_CORE, np.int64)
        np.add.at(cnt, blk, 1)
        per_core.append(dict(es=es, ed=ed, cnt=cnt))

    cnts = np.stack([pc["cnt"] for pc in per_core])  # [8, 49]
    C = (-(-cnts // BLK)).max(axis=0)  # [49] per-position chunk counts
    S = int(C.sum())

    in_maps = []
    for c in range(N_CORES):
        pc = per_core[c]
        es, ed, cnt = pc["es"], pc["ed"], pc["cnt"]
        # per-slot source column list, padded with the zero column
        col = np.full(S * BLK, N_PAD, np.int64)
        meta = np.zeros((128, S * 256), F8)  # per slot: M (128) | M^T (128)
        off = 0
        ptr = 0
        for b in range(BLOCKS_PER_CORE):
            Cc = int(C[b])
            if Cc == 0:
                continue
            n = int(cnt[b])
            col[off * BLK : off * BLK + n] = es[ptr : ptr + n]
            drel = ed[ptr : ptr + n] - b * BLK
            ptr += n
            for cc in range(Cc):
                lo = cc * BLK
                m = min(BLK, n - lo)
                if m <= 0:
                    break
                d = drel[lo : lo + m]
                base = (off + cc) * 256
                Mc = np.zeros((BLK, BLK), np.float32)
                Mc[d, np.arange(m)] = 1.0
                meta[:, base : base + 128] = Mc.astype(F8)
                MTc = np.zeros((BLK, BLK), np.float32)
                MTc[np.arange(m), d] = 1.0
                meta[:, base + 128 : base + 256] = MTc.astype(F8)
            off += Cc
        ze = np.ascontiguousarray(zT[:, col])  # [128, S*128] fp16
        zq = np.ascontiguousarray(
            zT[:, c * NODES_PER_CORE : c * NODES_PER_CORE + NODES_PER_CORE]
        )
        in_maps.append(
            dict(
                ze=ze,
                zq=zq,
                Wall=W_all.astype(np.float16),
                bias=b_all.reshape(1, 384).astype(np.float16),
                meta=meta,
            )
        )
    consts = dict(C=C, S=S, has_bias=has_bias)
    return in_maps, consts


def _build(consts):
    import concourse.bacc as bacc
    import concourse.mybir as mybir
    import concourse.tile as tile

    dt = mybir.dt
    Alu = mybir.AluOpType
    Act = mybir.ActivationFunctionType

    C = consts["C"]
    S = consts["S"]
    has_bias = consts["has_bias"]

    nc = bacc.Bacc("TRN2", target_bir_lowering=False, debug=False, num_devices=N_CORES)

    ze = nc.declare_dram_parameter("ze", [128, S * BLK], dt.float16, isOutput=False)
    zq = nc.declare_dram_parameter("zq", [128, NODES_PER_CORE], dt.float16, isOutput=False)
    Wall = nc.declare_dram_parameter("Wall", [128, 384], dt.float16, isOutput=False)
    bias = nc.declare_dram_parameter("bias", [1, 384], dt.float16, isOutput=False)
    meta = nc.declare_dram_parameter("meta", [128, S * 256], dt.float8e4, isOutput=False)
    h = nc.declare_dram_parameter("h", [NODES_PER_CORE, DIM], dt.float32, isOutput=True)

    with tile.TileContext(nc) as tc:
        with (
            tc.tile_pool(name="const", bufs=1) as constp,
            tc.tile_pool(name="qbuf", bufs=1) as qbuf,
        ):
            wall_sb = constp.tile([128, 384], dt.float16)
            nc.sync.dma_start(wall_sb[:], Wall[:])
            if has_bias:
                bias_sb = constp.tile([1, 384], dt.float16)
                ones1 = constp.tile([1, 128], dt.float16)
                nc.sync.dma_start(bias_sb[:], bias[:])
                nc.vector.memset(ones1[:], 1.0)
            q_sb = qbuf.tile([128, BLOCKS_PER_CORE * BLK], dt.float16)

            # ---- PE warm-up: ~9us of dense matmuls so the HAM clock-gate
            # lifts the PE from 1.2 to 2.4 GHz before the main loop ----
            with tc.tile_pool(name="warm", bufs=4, space="PSUM") as wpool:
                for i in range(80):
                    wps = wpool.tile([128, 128], dt.float32, tag="w")
                    nc.tensor.matmul(
                        wps[:], lhsT=wall_sb[:, 0:128], rhs=wall_sb[:, 0:128],
                        start=True, stop=True,
                    )

            # ---- prologue: project q for the core's own blocks ----
            with (
                tc.tile_pool(name="zt", bufs=3) as zpool,
                tc.tile_pool(name="pps", bufs=3, space="PSUM") as ppool,
            ):
                for g in range((BLOCKS_PER_CORE + 3) // 4):  # 4 blocks per DMA
                    lo = g * 4
                    nb = min(4, BLOCKS_PER_CORE - lo)
                    zt = zpool.tile([128, nb * 128], dt.float16, tag="zt")
                    nc.sync.dma_start(
                        zt[:], zq[:, lo * 128 : (lo + nb) * 128]
                    )
                    for i in range(nb):
                        b = lo + i
                        ps = ppool.tile([128, 128], dt.float32, tag="ps")
                        nc.tensor.matmul(
                            ps[:], lhsT=zt[:, i * 128 : (i + 1) * 128],
                            rhs=wall_sb[:, 0:128], start=True, stop=not has_bias,
                        )
                        if has_bias:
                            nc.tensor.matmul(
                                ps[:], lhsT=ones1[:], rhs=bias_sb[:, 0:128],
                                start=False, stop=True,
                            )
                        nc.scalar.copy(q_sb[:, b * 128 : (b + 1) * 128], ps[:])

            # ---- edge phase ----
            with (
                tc.tile_pool(name="zep", bufs=3) as zep,
                tc.tile_pool(name="mp", bufs=3) as mp,
                tc.tile_pool(name="kvp", bufs=10) as kvp,
                tc.tile_pool(name="xp", bufs=8) as xp,
                tc.tile_pool(name="ep", bufs=8) as ep,
                tc.tile_pool(name="scr", bufs=4) as scr,
                tc.tile_pool(name="hp", bufs=2) as hp,
                tc.tile_pool(name="sm", bufs=4) as sm,
                tc.tile_pool(name="pskv", bufs=3, space="PSUM") as pskv,
                tc.tile_pool(name="psq", bufs=3, space="PSUM") as psq,
                tc.tile_pool(name="psnd", bufs=2, space="PSUM") as psnd,
            ):
                offs = np.concatenate([[0], np.cumsum(C)]).astype(int)
                for b in range(BLOCKS_PER_CORE):
                    Cc = int(C[b])
                    off = int(offs[b])
                    nd = psnd.tile([128, 132], dt.float32, tag="nd")
                    ht = hp.tile([128, 128], dt.float32, tag="h")
                    if Cc == 0:
                        nc.vector.memset(ht[:], 0.0)
                        nc.sync.dma_start(h[b * 128 : (b + 1) * 128, :], ht[:])
                        continue
                    zs = zep.tile([128, Cc * 128], dt.float16, tag="ze")
                    nc.sync.dma_start(zs[:], ze[:, off * 128 : (off + Cc) * 128])
                    mt_ = mp.tile([128, Cc * 256], dt.float8e4, tag="m")
                    nc.sync.dma_start(mt_[:], meta[:, off * 256 : (off + Cc) * 256])
                    for c0 in range(0, Cc, 4):
                        npair = min(4, Cc - c0)
                        xt2 = xp.tile([128, 4, 132], dt.float16, tag="x")
                        e2 = ep.tile([128, 4], dt.float32, tag="e")
                        kvts = []
                        for j in range(npair):
                            cc = c0 + j
                            # per-edge k|v projection
                            kvps = pskv.tile([128, 256], dt.float32, tag="kvps")
                            nc.tensor.matmul(
                                kvps[:], lhsT=zs[:, cc * 128 : (cc + 1) * 128],
                                rhs=wall_sb[:, 128:384], start=True, stop=not has_bias,
                            )
                            if has_bias:
                                nc.tensor.matmul(
                                    kvps[:], lhsT=ones1[:], rhs=bias_sb[:, 128:384],
                                    start=False, stop=True,
                                )
                            kvt = kvp.tile([128, 256], dt.float16, tag="kvt")
                            if cc % 3 == 0:
                                nc.vector.tensor_copy(kvt[:], kvps[:])
                            else:
                                nc.scalar.copy(kvt[:], kvps[:])
                            kvts.append(kvt)
                            # q expansion to edges
                            qd = psq.tile([128, 128], dt.float32, tag="qd")
                            nc.tensor.matmul(
                                qd[:],
                                lhsT=mt_[:, cc * 256 : cc * 256 + 128],
                                rhs=q_sb[:, b * 128 : (b + 1) * 128],
                                start=True, stop=True,
                            )
                            # e = tau * sum_f k*q_dst  (fused mul+reduce)
                            sct = scr.tile([128, 128], dt.float16, tag="sc")
                            nc.vector.affine_mul_reduce(
                                out=sct[:], accum_out=e2[:, j : j + 1],
                                in0=kvt[:, 0:128], in1=qd[:],
                                scale=float(TAU), bias=0.0,
                            )
                        # one exp per chunk pair, strided into the den column
                        nc.scalar.activation(
                            xt2[:, 0:npair, 128:129],
                            e2[:, 0:npair].rearrange("p (a b) -> p a b", b=1),
                            Act.Exp,
                        )
                        for j in range(npair):
                            cc = c0 + j
                            # x = v * exp(e) on the otherwise-idle GPSIMD engine
                            nc.gpsimd.tensor_tensor(
                                out=xt2[:, j, 0:128],
                                in0=kvts[j][:, 128:256],
                                in1=xt2[:, j, 128:129].broadcast_to((128, 128)),
                                op=Alu.mult,
                            )
                            # segment-sum into num|den
                            nc.tensor.matmul(
                                nd[:, 0:129],
                                lhsT=mt_[:, cc * 256 + 128 : cc * 256 + 256],
                                rhs=xt2[:, j, 0:129],
                                start=(cc == 0), stop=(cc == Cc - 1),
                            )
                    # ---- drain block b: h = num / max(den, den==0) ----
                    z01 = sm.tile([128, 1], dt.float32, tag="z01")
                    nc.vector.tensor_scalar(
                        out=z01[:], in0=nd[:, 128:129],
                        scalar1=0.0, scalar2=None, op0=Alu.is_equal,
                    )
                    dsafe = sm.tile([128, 1], dt.float32, tag="ds")
                    nc.vector.tensor_tensor(
                        out=dsafe[:], in0=nd[:, 128:129], in1=z01[:], op=Alu.add
                    )
                    rec = sm.tile([128, 1], dt.float32, tag="rec")
                    nc.vector.reciprocal(rec[:], dsafe[:])
                    nc.vector.tensor_scalar(
                        out=ht[:], in0=nd[:, 0:128],
                        scalar1=rec[:], scalar2=None, op0=Alu.mult,
                    )
                    nc.sync.dma_start(h[b * 128 : (b + 1) * 128, :], ht[:])

    nc.compile()
    return nc


def _install_ntff_hook():
    """The agent image's antenv lacks axon_hooks; recreate it and register
    the ctypes NTFF profile hook the boot would have installed."""
    import types

    if "antenv.axon_hooks" not in sys.modules:
        import antenv

        m = types.ModuleType("antenv.axon_hooks")
        m._hook = None
        m.set_axon_ntff_profile_hook = lambda h, _m=m: setattr(_m, "_hook", h)
        m.get_axon_ntff_profile_hook = lambda _m=m: _m._hook
        sys.modules["antenv.axon_hooks"] = m
        antenv.axon_hooks = m
    from antenv import axon_hooks

    if axon_hooks.get_axon_ntff_profile_hook() is None:
        from trn_agent_boot.trn_boot import _ntff_profile_via_ctypes

        hook = _ntff_profile_via_ctypes("/opt/axon/libaxon_pjrt.so")
        if hook is not None:
            axon_hooks.set_axon_ntff_profile_hook(hook)


def run(inputs, trace=False):
    """Returns (h [50000,128] float32, exec_time_ns or None)."""
    from concourse.bass_utils import run_bass_kernel_spmd

    if trace:
        try:
            _install_ntff_hook()
        except Exception as e:  # profiling is best-effort
            print(f"ntff hook install failed: {e}", file=sys.stderr)

    in_maps, consts = _prepare(**inputs)
    nc = _build(consts)
    res = run_bass_kernel_spmd(
        nc,
        [dict(m) for m in in_maps],
        list(range(N_CORES)),
        trace=trace,
    )
    h = np.concatenate([r["h"] for r in res.results], axis=0)[:N_NODES]
    return np.ascontiguousarray(h.astype(np.float32)), res.exec_time_ns


def kernel(**inputs) -> np.ndarray:
    h, _ = run(inputs, trace=False)
    return h
